# revision 5
# baseline (speedup 1.0000x reference)
"""OIM loss kernel for Trainium2, 8 NeuronCores, data-parallel over the roi dim.

Math (per reference):
    bank   = concat([lut, cq], 0)                      # [L=10532, D=256]
    logits = (inputs @ bank.T) * reliability * 30.0    # [N=8192, L]
    loss   = mean over rows with label != 5554 of
             logsumexp(logits[r]) - logits[r, label[r]]

Distribution: rows split 1024/core across 8 cores; the (reliability*30)-scaled
bank is replicated in fp8-e4m3 (inputs are pre-scaled x16 so both fp8 operands
sit near unit variance; the exp() folds the 1/16 back in via its scale field).
Each core returns [sum of masked nll, n_valid]; the host sums the 8 partials.

Per-core pipeline, all five engines in play (the softmax denominator needs one
exp pass + one row-sum pass over 10.5K x 1K logits; one engine alone paces at
~80us, so the passes are split by a static per-block path table):
  PE : fp8 DoubleRow matmuls (2 cols/cycle), one explicit LDWEIGHTS per
       (row-tile, col-block), non-self-loading matmuls dep-chained behind it
  ACT: exp over most col-blocks (scale=1/16 fused), some with fused row-sum
       accumulator
  DVE: Schraudolph bit-trick exp for some blocks (affine f32->int16 write,
       reinterpreted as bf16), row-sum reduces, picked-logit dots, bitcast
       log-approx for ln(sumexp) (avoids the ACT Ln-table swap)
  GPS: row-sums via tensor_scalar bypass+accum (gpsimd cannot reduce axis-X
       nor read PSUM, but can accumulate an SBUF pass)
Block paths: A = ACT exp + ACT accum | B = ACT exp + DVE sum
             C = ACT exp + GPS sum   | D = DVE bit-exp + GPS sum
             E = DVE bit-exp + DVE sum
"""

import numpy as np
import ml_dtypes

N = 8192
D = 256
L = 10532  # 5532 + 5000
NCORES = 8
NSH = N // NCORES     # 1024 rows per core
P = 128               # partitions
RT = NSH // P         # 8 row tiles per core
KC = D // P           # 2 contraction chunks (DoubleRow pair)
IGNORE = 5554
OIM_SCALAR = 30.0
FP8_SCALE = 16.0      # inputs pre-scaled by this; exp() divides it back out

# col-blocks: small first blocks start the exp pipeline while banks stream in
WIDTHS = [1024, 1024, 2048, 2048, 2048, 2048, 292]
OFFS = [sum(WIDTHS[:i]) for i in range(len(WIDTHS))]
NCB = len(WIDTHS)
assert sum(WIDTHS) == L

# per-block engine path table [cb][rt] (see module docstring)
PATHS = [
    "BEBEBEBE",  # cb0 1024-wide: DVE is idle early, give it bit-exp work
    "CBCBEBCB",  # cb1 1024-wide
    "ABCDBABA",  # cb2..cb5 2048-wide: mixed to balance ACT/DVE/GPS
    "ABCDBABA",
    "ABCDBABA",
    "ABCDBABA",
    "BBABBABB",  # cb6 292-wide: cheap
]

# Schraudolph exp on bf16 bit patterns: i16 = trunc(a*raw + b) viewed as bf16
# approximates exp(raw/16).  b tuned so block sums are unbiased under the
# truncating float->int convert (numpy/CoreSim; hw round-to-nearest shifts
# exp by +0.27% which is still far inside tolerance).
SCHRAU_A = 128.0 / np.log(2.0) / FP8_SCALE
SCHRAU_B = 16249.136
# ln(x) ~= float_bits_as_int(x) * ln2/2^23 - C  (same trick in reverse)
FLN_K = float(np.log(2.0) / 2**23)
FLN_C = 88.02637566918142

BF16 = ml_dtypes.bfloat16
FP8 = ml_dtypes.float8_e4m3

_CACHE = {}


def _build(debug=False):
    import concourse.bacc as bacc
    import concourse.tile as tile
    from concourse import mybir

    fp8 = mybir.dt.float8e4
    bf16 = mybir.dt.bfloat16
    f32 = mybir.dt.float32
    i16 = mybir.dt.int16
    i32 = mybir.dt.int32
    AF = mybir.ActivationFunctionType
    ALU = mybir.AluOpType
    AX = mybir.AxisListType
    DR = mybir.MatmulPerfMode.DoubleRow

    nc = bacc.Bacc(
        "TRN2", target_bir_lowering=False, debug=debug, enable_partition_id=False
    )

    # element (p, rt, k, c) = x16[rt*128 + c, k*128 + p]
    d_inp = nc.dram_tensor("inp", [P, RT, KC, P], fp8, kind="ExternalInput").ap()
    # element (p, k, j) = scaled[j, k*128 + p]
    d_bank = nc.dram_tensor("bank", [P, KC, L], fp8, kind="ExternalInput").ap()
    d_rows = nc.dram_tensor("rows", [P, RT, D], bf16, kind="ExternalInput").ap()
    d_bsel = nc.dram_tensor("bsel", [P, RT, D], bf16, kind="ExternalInput").ap()
    d_mask = nc.dram_tensor("mask", [P, RT], f32, kind="ExternalInput").ap()
    d_out = nc.dram_tensor("out", [1, 2], f32, kind="ExternalOutput").ap()

    with tile.TileContext(nc) as tc:
        with (
            tc.tile_pool(name="const", bufs=1) as const,
            tc.tile_pool(name="work", bufs=2) as work,
            tc.tile_pool(name="psum", bufs=2, space="PSUM") as psum,
        ):
            # --- resident inputs ---
            inp_sb = const.tile([P, RT, KC, P], fp8)
            bank_sb = [
                const.tile([P, KC, WIDTHS[cb]], fp8, tag=f"bk{cb}", name=f"bk{cb}")
                for cb in range(NCB)
            ]
            rows_sb = const.tile([P, RT, D], bf16)
            bsel_sb = const.tile([P, RT, D], bf16)
            mask_sb = const.tile([P, RT], f32)

            # --- startup DMA: critical pieces fan out on 4 engine queues ---
            # sync: weights head + first half of col-block 0
            nc.sync.dma_start(out=inp_sb[:, 0], in_=d_inp[:, 0])
            nc.sync.dma_start(
                out=bank_sb[0][:, :, 0:512], in_=d_bank[:, :, 0:512]
            )
            # scalar: second half of col-block 0 + weights tail (ACT idle
            # until the first block lands anyway)
            nc.scalar.dma_start(
                out=bank_sb[0][:, :, 512:1024], in_=d_bank[:, :, 512:1024]
            )
            nc.scalar.dma_start(out=inp_sb[:, 1:], in_=d_inp[:, 1:])
            # gpsimd: col-block 1 now; later banks gated below
            nc.gpsimd.dma_start(
                out=bank_sb[1], in_=d_bank[:, :, OFFS[1] : OFFS[1] + 1024]
            )
            late_dmas = []  # (anchor exp idx, inst)
            for cb in range(2, NCB):
                anchor = (cb - 2) * 8 + 2
                late_dmas.append(
                    (
                        anchor,
                        nc.gpsimd.dma_start(
                            out=bank_sb[cb],
                            in_=d_bank[:, :, OFFS[cb] : OFFS[cb] + WIDTHS[cb]],
                        ),
                    )
                )
            late_dmas.append((8, nc.sync.dma_start(out=rows_sb, in_=d_rows)))
            late_dmas.append((10, nc.sync.dma_start(out=bsel_sb, in_=d_bsel)))
            late_dmas.append((12, nc.sync.dma_start(out=mask_sb, in_=d_mask)))

            # --- ACT exp-table preload: tiny dummy exp scheduled first ---
            tiny = const.tile([P, 1], f32)
            nc.vector.memset(tiny, 0.0)
            tiny_o = const.tile([P, 1], f32)
            nc.scalar.activation(out=tiny_o, in_=tiny, func=AF.Exp)

            # --- PE warmup: ramp the HAM clock gate during the DMA wait ---
            wsrc = const.tile([P, KC, 512], fp8)
            nc.vector.memset(wsrc, 0.25)
            pw = psum.tile([P, 2048], f32, tag="ps", name="warm")
            warm_mms = []
            for i in range(8):
                m = nc.tensor.matmul(
                    pw[:, 0:512],
                    wsrc[:, :, 0:P],
                    wsrc,
                    start=True,
                    stop=True,
                    perf_mode=DR,
                )
                if warm_mms:
                    tile.add_dep_helper(m.ins, warm_mms[-1].ins, reason="warm order")
                warm_mms.append(m)

            # --- picked logit: dot(inputs[r], scaled_bank[label[r]]) on DVE ---
            picked = const.tile([P, RT], f32)
            dots = const.tile([P, RT, D], f32)
            for rt in range(RT):
                nc.vector.tensor_mul(
                    dots[:, rt, :], rows_sb[:, rt, :], bsel_sb[:, rt, :]
                )
                nc.vector.tensor_reduce(
                    out=picked[:, rt : rt + 1],
                    in_=dots[:, rt, :],
                    axis=AX.X,
                    op=ALU.add,
                )

            def gps_tree(es, w, acc):
                """Row-sum on gpsimd: in-place add tree w -> 16, DVE finishes.
                (Pool can't reduce axis-X; tensor_tensor add is its one legal
                high-throughput ALU op.)"""
                hw_ = w // 2
                while hw_ >= 16:
                    nc.gpsimd.tensor_tensor(
                        es[:, :hw_], es[:, :hw_], es[:, hw_ : 2 * hw_], op=ALU.add
                    )
                    hw_ //= 2
                nc.vector.tensor_reduce(
                    out=acc, in_=es[:, :16], axis=AX.X, op=ALU.add
                )

            # --- main loop: logits blocks -> exp -> row sums, path per block ---
            blocksums = const.tile([P, RT * NCB], f32)
            es_big = work.tile([P, 8, 2048], bf16, bufs=1)
            nes = 0
            exps = []       # per-block "exp is done" instr, for DMA anchors
            prev_mms = warm_mms
            for cb in range(NCB):
                w = WIDTHS[cb]
                nb = (w + 511) // 512
                for rt in range(RT):
                    path = PATHS[cb][rt]
                    ps = psum.tile([P, 2048], f32, tag="ps", name=f"ps_{cb}_{rt}")
                    lhsT = inp_sb[:, rt]
                    ldw = nc.tensor.ldweights(lhsT, perf_mode=DR)
                    for pm in prev_mms:
                        tile.add_dep_helper(
                            ldw.ins, pm.ins, reason="ldweights after prev block mms"
                        )
                    mms = []
                    for b in range(nb):
                        bw = min(512, w - b * 512)
                        m = nc.tensor.matmul(
                            ps[:, b * 512 : b * 512 + bw],
                            lhsT,
                            bank_sb[cb][:, :, b * 512 : b * 512 + bw],
                            start=True,
                            stop=True,
                            perf_mode=DR,
                        )
                        m.ins.ldweights = False
                        tile.add_dep_helper(
                            m.ins, ldw.ins, reason="mm after its ldweights"
                        )
                        mms.append(m)
                    prev_mms = mms

                    acc = blocksums[:, rt * NCB + cb : rt * NCB + cb + 1]
                    if path == "A":
                        a = nc.scalar.activation(
                            out=ps[:, :w],
                            in_=ps[:, :w],
                            func=AF.Exp,
                            scale=1.0 / FP8_SCALE,
                            accum_out=acc,
                        )
                    elif path in ("B", "C"):
                        es = es_big[:, nes % 8, :w]
                        nes += 1
                        a = nc.scalar.activation(
                            out=es, in_=ps[:, :w], func=AF.Exp, scale=1.0 / FP8_SCALE
                        )
                        if path == "B":
                            nc.vector.tensor_reduce(
                                out=acc, in_=es, axis=AX.X, op=ALU.add
                            )
                        else:
                            gps_tree(es, w, acc)
                    else:  # D / E: Schraudolph bit-exp on DVE
                        es = es_big[:, nes % 8, :w]
                        nes += 1
                        a = nc.vector.tensor_scalar(
                            out=es.bitcast(i16),
                            in0=ps[:, :w],
                            scalar1=SCHRAU_A,
                            scalar2=SCHRAU_B,
                            op0=ALU.mult,
                            op1=ALU.add,
                        )
                        if path == "D":
                            gps_tree(es, w, acc)
                        else:
                            nc.vector.tensor_reduce(
                                out=acc, in_=es, axis=AX.X, op=ALU.add
                            )
                    exps.append(a)
            for anchor, dma in late_dmas:
                tile.add_dep_helper(
                    dma.ins,
                    exps[anchor].ins,
                    reason="hold non-critical DMAs off the startup window",
                )

            # --- tail: nll = ln(sumexp) - picked, masked sums (no ACT) ---
            sumexp = const.tile([P, RT], f32)
            nc.vector.tensor_reduce(
                out=sumexp,
                in_=blocksums.rearrange("p (r c) -> p r c", c=NCB),
                axis=AX.X,
                op=ALU.add,
            )
            lnse = const.tile([P, RT], f32)
            nc.vector.tensor_scalar(
                out=lnse,
                in0=sumexp.bitcast(i32),
                scalar1=FLN_K,
                scalar2=FLN_C,
                op0=ALU.mult,
                op1=ALU.subtract,
            )
            nll = const.tile([P, RT], f32)
            nc.vector.tensor_sub(nll, lnse, picked)
            masked = const.tile([P, RT], f32)
            nc.vector.tensor_mul(masked, nll, mask_sb)

            stacked = const.tile([P, 2], f32)
            nc.vector.tensor_reduce(
                out=stacked[:, 0:1], in_=masked, axis=AX.X, op=ALU.add
            )
            nc.vector.tensor_reduce(
                out=stacked[:, 1:2], in_=mask_sb, axis=AX.X, op=ALU.add
            )

            ones = const.tile([P, 1], f32)
            nc.vector.memset(ones, 1.0)
            fin = psum.tile([P, 2048], f32, tag="ps", name="fin")
            nc.tensor.matmul(fin[0:1, 0:2], ones, stacked, start=True, stop=True)
            out_sb = const.tile([1, 2], f32)
            nc.vector.tensor_copy(out=out_sb, in_=fin[0:1, 0:2])
            nc.sync.dma_start(out=d_out, in_=out_sb)

    nc.compile()
    return nc


def get_nc(debug=False):
    key = ("nc", debug)
    if key not in _CACHE:
        _CACHE[key] = _build(debug=debug)
    return _CACHE[key]


def make_in_maps(inputs, label, ious, lut, cq, reliability):
    """Host-side shard prep. Index gathers / transposes / casts only."""
    inputs = np.asarray(inputs, dtype=np.float32)
    label = np.asarray(label).astype(np.int64)
    lut = np.asarray(lut, dtype=np.float32)
    cq = np.asarray(cq, dtype=np.float32)
    reliability = np.asarray(reliability, dtype=np.float32)

    bank = np.concatenate([lut, cq], axis=0)                 # [L, D]
    scaled = bank * (OIM_SCALAR * reliability)[:, None]      # [L, D] fp32
    # [P, KC, L] fp8: (p, k, j) = scaled[j, k*128+p]
    bank8 = np.ascontiguousarray(
        scaled.T.reshape(KC, P, L).transpose(1, 0, 2)
    ).astype(FP8)

    valid = label != IGNORE
    safe = np.where(valid, label, 0)
    bsel_full = scaled[safe].astype(BF16)                    # [N, D]
    inp_bf = inputs.astype(BF16)                             # [N, D]
    inp8_full = (inputs * FP8_SCALE).astype(FP8)             # [N, D]

    in_maps = []
    for c in range(NCORES):
        sl = slice(c * NSH, (c + 1) * NSH)
        # [P, RT, KC, P]: (p, rt, k, c) = x16[rt*128+c, k*128+p]
        x8 = inp8_full[sl].astype(FP8)
        inp = np.ascontiguousarray(
            x8.T.reshape(KC, P, RT, P).transpose(1, 2, 0, 3)
        )
        x = inp_bf[sl]
        rows = np.ascontiguousarray(x.reshape(RT, P, D).transpose(1, 0, 2))
        bsel = np.ascontiguousarray(
            bsel_full[sl].reshape(RT, P, D).transpose(1, 0, 2)
        )
        mask = np.ascontiguousarray(
            valid[sl].reshape(RT, P).T.astype(np.float32)
        )
        in_maps.append(
            {"inp": inp, "bank": bank8, "rows": rows, "bsel": bsel, "mask": mask}
        )
    return in_maps


def _combine(parts):
    """parts: list of [1,2] arrays per core -> scalar loss."""
    arr = np.stack([np.asarray(p, dtype=np.float64) for p in parts])  # [8,1,2]
    total = arr[:, 0, 0].sum()
    count = arr[:, 0, 1].sum()
    return np.float32(total / max(count, 1.0))


def kernel(inputs, label, ious, lut, cq, reliability):
    from concourse import bass_utils

    nc = get_nc()
    in_maps = make_in_maps(inputs, label, ious, lut, cq, reliability)
    res = bass_utils.run_bass_kernel_spmd(nc, in_maps, core_ids=list(range(NCORES)))
    return _combine([r["out"] for r in res.results])
